# revision 1
# baseline (speedup 1.0000x reference)
"""Trainium2 Bass kernel for nn_BilinearBlock (bilinear attention + bilinear MLP block).

Sharding: 8 cores = (batch b in 0..3) x (sequence half h in 0..1).
Each core computes output rows [h*1024, (h+1)*1024) of batch b.

Everything on-device is kept feature-major ("T layout": features/head-dims on
SBUF partitions, sequence positions on the free axis) so that every matmul
contracts over the partition dim with zero on-device transposes of
activations (only V needs a PE transpose).  RMSNorm is algebraically commuted
past the linear projections: projections run on raw x, and the per-row norm
factor r = rsqrt(mean(x^2)+eps) is applied to the small projected tensors.
The causal mask (an arbitrary 0/1 mask, taken from the causal_mask input) is
applied as a multiply. All matmuls run in float32r (full PE rate, ~2^-13
rounding).
"""
import os
import sys

for _p in ('/opt/trn_rl_repo',):
    if _p not in sys.path:
        sys.path.insert(0, _p)

import numpy as np
import ml_dtypes

import concourse.bass as bass
import concourse.mybir as mybir
import concourse.tile as tile
from concourse import bacc
from concourse.bass_utils import run_bass_kernel_spmd
from concourse.masks import make_identity

P = 128
S = 2048          # full sequence
R = 1024          # query rows per core
D = 1024          # d_model
DH = 128          # d_head
DM = 4096         # d_mlp
NT = 512          # matmul moving free dim
FC = D // P       # 8 feature chunks
TC = S // P       # 16 t chunks
NGRP = 4          # d_mlp groups for the wp pass
GK = DM // P // NGRP  # 8 dm chunks per group
EPS = 1e-6
F32 = mybir.dt.float32
F32R = mybir.dt.float32r

LAST_EXEC_NS = None

_cached = {}


def _build():
    nc = bacc.Bacc("TRN2", target_bir_lowering=False, debug=False, num_devices=8)

    xT = nc.dram_tensor("xT", [D, S], F32R, kind="ExternalInput").ap()
    xqT = nc.dram_tensor("xqT", [D, R], F32R, kind="ExternalInput").ap()
    cos_kv = nc.dram_tensor("cos_kv", [DH, S], F32, kind="ExternalInput").ap()
    sin_kv = nc.dram_tensor("sin_kv", [DH, S], F32, kind="ExternalInput").ap()
    cos_q = nc.dram_tensor("cos_q", [DH, R], F32, kind="ExternalInput").ap()
    sin_q = nc.dram_tensor("sin_q", [DH, R], F32, kind="ExternalInput").ap()
    maskT = nc.dram_tensor("maskT", [S, R], mybir.dt.bfloat16, kind="ExternalInput").ap()
    wq1 = nc.dram_tensor("wq1", [D, DH], F32R, kind="ExternalInput").ap()
    wq2 = nc.dram_tensor("wq2", [D, DH], F32R, kind="ExternalInput").ap()
    wk1 = nc.dram_tensor("wk1", [D, DH], F32R, kind="ExternalInput").ap()
    wk2 = nc.dram_tensor("wk2", [D, DH], F32R, kind="ExternalInput").ap()
    wv = nc.dram_tensor("wv", [D, DH], F32R, kind="ExternalInput").ap()
    wo = nc.dram_tensor("wo", [DH, D], F32R, kind="ExternalInput").ap()
    wm = nc.dram_tensor("wm", [D, DM], F32R, kind="ExternalInput").ap()
    wn = nc.dram_tensor("wn", [D, DM], F32R, kind="ExternalInput").ap()
    wp = nc.dram_tensor("wp", [DM, D], F32R, kind="ExternalInput").ap()
    outT = nc.dram_tensor("outT", [D, R], F32, kind="ExternalOutput").ap()

    # DRAM scratch for broadcasting per-row norm factors across partitions
    rkv_d = nc.dram_tensor("rkv_scratch", [1, S], F32).ap()
    rq_d = nc.dram_tensor("rq_scratch", [1, R], F32).ap()
    r2_d = nc.dram_tensor("r2_scratch", [1, R], F32).ap()

    def bcast(src_dram):
        return bass.AP(tensor=src_dram.tensor, offset=src_dram.offset,
                       ap=[[0, P]] + list(src_dram.ap[1:]))

    with tile.TileContext(nc) as tc:
        with tc.tile_pool(name="glob", bufs=1) as glob, \
             tc.tile_pool(name="tmp", bufs=2) as tmp:

            ident = glob.tile([P, P], F32, tag="ident")
            make_identity(nc, ident)
            ones_f = glob.tile([P, 1], F32, tag="ones_f")
            nc.vector.memset(ones_f, 1.0)
            ones = glob.tile([P, 1], F32R, tag="ones")
            nc.vector.tensor_copy(out=ones, in_=ones_f)
            eps_t = glob.tile([1, 1], F32, tag="eps")
            nc.vector.memset(eps_t, EPS)
            out1T = [glob.tile([P, R], F32R, tag=f"out1T{f}", name=f"out1T{f}")
                     for f in range(FC)]

            with tc.tile_pool(name="attn", bufs=1) as attn:
                k1Tb = [attn.tile([DH, NT], F32R, tag=f"k1T{j}", name=f"k1T{j}")
                        for j in range(S // NT)]
                k2Tb = [attn.tile([DH, NT], F32R, tag=f"k2T{j}", name=f"k2T{j}")
                        for j in range(S // NT)]
                q1Tb = [attn.tile([DH, NT], F32R, tag=f"q1T{j}", name=f"q1T{j}")
                        for j in range(R // NT)]
                q2Tb = [attn.tile([DH, NT], F32R, tag=f"q2T{j}", name=f"q2T{j}")
                        for j in range(R // NT)]
                v_rm = [attn.tile([P, DH], F32R, tag=f"vrm{i}", name=f"vrm{i}")
                        for i in range(TC)]
                attnT = attn.tile([DH, R], F32R, tag="attnT")

                # ================= phase A: projections, block-streamed =========
                with tc.tile_pool(name="xs", bufs=2) as xs, \
                     tc.tile_pool(name="wks", bufs=1) as wks, \
                     tc.tile_pool(name="sc", bufs=2) as sc, \
                     tc.tile_pool(name="psA", bufs=2, space="PSUM") as psA:

                    wblks = {}
                    for nm, w in [("wq1", wq1), ("wq2", wq2), ("wk1", wk1),
                                  ("wk2", wk2), ("wv", wv)]:
                        t = wks.tile([P, FC, DH], F32R, tag=nm, name=nm)
                        nc.gpsimd.dma_start(
                            out=t, in_=w.rearrange("(ko p) m -> p ko m", p=P))
                        wblks[nm] = t

                    def do_block(x_dram, r_dram, cos_d, sin_d, sl, projs, tbase,
                                 xtag="xb", xbufs=2):
                        """Process one 512-column block: norm factor + projections.

                        projs: list of (wname, out_tile or vrm handling, kind)
                        """
                        xr = x_dram.rearrange("(ko p) n -> p ko n", p=P)
                        xb = []
                        for f in range(FC):
                            t = xs.tile([P, NT], F32R, tag=f"{xtag}{f}",
                                        name=f"{xtag}{f}", bufs=xbufs)
                            nc.sync.dma_start(out=t, in_=xr[:, f, sl])
                            xb.append(t)
                        # norm factor for this block
                        rp = psA.tile([1, NT], F32, tag="rp", bufs=1)
                        for f in range(FC):
                            sq = tmp.tile([P, NT], F32R, tag="sqr")
                            sf = xb[f].bitcast(F32)
                            if f % 2 == 0:
                                nc.scalar.activation(
                                    out=sq, in_=sf,
                                    func=mybir.ActivationFunctionType.Square,
                                    bias=0.0, scale=1.0)
                            else:
                                nc.vector.tensor_mul(out=sq, in0=sf, in1=sf)
                            nc.tensor.matmul(rp, ones, sq,
                                             start=(f == 0), stop=(f == FC - 1))
                        rsb = tmp.tile([1, NT], F32, tag="rsb")
                        nc.scalar.activation(out=rsb, in_=rp,
                                             func=mybir.ActivationFunctionType.Sqrt,
                                             bias=eps_t, scale=1.0 / D)
                        rsb2 = tmp.tile([1, NT], F32, tag="rsb2")
                        nc.vector.reciprocal_approx_fast(out=rsb2, in_=rsb)
                        nc.gpsimd.dma_start(out=r_dram[:, sl], in_=rsb2)
                        rbb = xs.tile([P, NT], F32, tag="rbb")
                        nc.gpsimd.dma_start(out=rbb, in_=bcast(r_dram[:, sl]))
                        # rope tables for this block
                        cosb = xs.tile([DH, NT], F32, tag="cosb")
                        nc.sync.dma_start(out=cosb, in_=cos_d[:, sl])
                        sinb = xs.tile([DH, NT], F32, tag="sinb")
                        nc.sync.dma_start(out=sinb, in_=sin_d[:, sl])

                        for wname, dst, kind in projs:
                            pp = psA.tile([P, NT], F32, tag="pp", bufs=4)
                            wb = wblks[wname]
                            for f in range(FC):
                                nc.tensor.matmul(pp, wb[:, f], xb[f],
                                                 start=(f == 0), stop=(f == FC - 1))
                            if kind == "rope":
                                t1 = tmp.tile([P, NT], F32, tag="t1")
                                nc.vector.tensor_mul(out=t1, in0=pp, in1=cosb)
                                rot = tmp.tile([P, NT], F32, tag="rot")
                                nc.scalar.activation(
                                    out=rot[0:64], in_=pp[64:128],
                                    func=mybir.ActivationFunctionType.Copy,
                                    bias=0.0, scale=1.0)
                                nc.scalar.activation(
                                    out=rot[64:128], in_=pp[0:64],
                                    func=mybir.ActivationFunctionType.Copy,
                                    bias=0.0, scale=1.0)
                                nc.vector.tensor_mul(out=rot, in0=rot, in1=sinb)
                                nc.vector.tensor_add(out=t1, in0=t1, in1=rot)
                                nc.vector.tensor_mul(out=dst, in0=t1, in1=rbb)
                            else:  # v: scale + transpose to row-major blocks
                                vt = tmp.tile([P, NT], F32, tag="t1")
                                nc.vector.tensor_mul(out=vt, in0=pp, in1=rbb)
                                for t in range(NT // P):
                                    tp = psA.tile([P, P], F32, tag="tp", bufs=1)
                                    nc.tensor.transpose(tp, vt[:, t * P:(t + 1) * P],
                                                        ident)
                                    nc.scalar.activation(
                                        out=v_rm[tbase + t], in_=tp,
                                        func=mybir.ActivationFunctionType.Copy,
                                        bias=0.0, scale=1.0)

                    for jb in range(R // NT):
                        sl = slice(jb * NT, (jb + 1) * NT)
                        do_block(xqT, rq_d, cos_q, sin_q, sl,
                                 [("wq1", q1Tb[jb], "rope"),
                                  ("wq2", q2Tb[jb], "rope")],
                                 tbase=0, xtag="xq", xbufs=2)
                    for jb in range(S // NT):
                        sl = slice(jb * NT, (jb + 1) * NT)
                        do_block(xT, rkv_d, cos_kv, sin_kv, sl,
                                 [("wk1", k1Tb[jb], "rope"),
                                  ("wk2", k2Tb[jb], "rope"),
                                  ("wv", None, "v")], tbase=jb * (NT // P))

                    # ---- scores + attn@v, interleaved with projections ----
                    avp = [psA.tile([P, NT], F32, tag=f"av{hj}", name=f"av{hj}",
                                    bufs=1)
                           for hj in range(R // NT)]
                    for i in range(TC):
                        mk = sc.tile([P, R], mybir.dt.bfloat16, tag="mk")
                        nc.sync.dma_start(out=mk, in_=maskT[i * P:(i + 1) * P, :])
                        kb, ko = i // 4, (i % 4) * P
                        for hj in range(R // NT):
                            s1 = psA.tile([P, NT], F32, tag="pp", name="s1", bufs=4)
                            nc.tensor.matmul(s1, k1Tb[kb][:, ko:ko + P],
                                             q1Tb[hj], start=True, stop=True)
                            s2 = psA.tile([P, NT], F32, tag="pp", name="s2", bufs=4)
                            nc.tensor.matmul(s2, k2Tb[kb][:, ko:ko + P],
                                             q2Tb[hj], start=True, stop=True)
                            sm = tmp.tile([P, NT], F32, tag="sm", bufs=3)
                            nc.vector.tensor_mul(out=sm, in0=s1,
                                                 in1=mk[:, hj * NT:(hj + 1) * NT])
                            aT = sc.tile([P, NT], F32R, tag="aT", bufs=4)
                            nc.vector.tensor_mul(out=aT, in0=sm, in1=s2)
                            nc.tensor.matmul(avp[hj], v_rm[i], aT,
                                             start=(i == 0), stop=(i == TC - 1))
                    for hj in range(R // NT):
                        nc.vector.tensor_copy(
                            out=attnT[:, hj * NT:(hj + 1) * NT], in_=avp[hj])


                # ============ phase C: out1 = x + attn @ wo ====================
                with tc.tile_pool(name="oc", bufs=2) as oc, \
                     tc.tile_pool(name="psC", bufs=2, space="PSUM") as psC:
                    woblk = oc.tile([P, FC, P], F32R, tag="wo", bufs=1)
                    nc.gpsimd.dma_start(
                        out=woblk, in_=wo.rearrange("d (ko m) -> d ko m", m=P))
                    for f in range(FC):
                        xqr = oc.tile([P, R], F32, tag="xqr")
                        nc.gpsimd.dma_start(
                            out=xqr, in_=xqT.bitcast(F32)[f * P:(f + 1) * P, :])
                        for hj in range(R // NT):
                            sl = slice(hj * NT, (hj + 1) * NT)
                            pw = psC.tile([P, NT], F32, tag="pw")
                            nc.tensor.matmul(pw, woblk[:, f], attnT[:, sl],
                                             start=True, stop=True)
                            nc.vector.tensor_add(out=out1T[f][:, sl], in0=pw,
                                                 in1=xqr[:, sl])

            # ================ phase D: rmsnorm2 + bilinear MLP =================
            with tc.tile_pool(name="mlp", bufs=1) as mlp, \
                 tc.tile_pool(name="ws", bufs=2) as ws, \
                 tc.tile_pool(name="tmpd", bufs=2) as tmpd, \
                 tc.tile_pool(name="psD", bufs=2, space="PSUM") as psD:

                nsl = R // NT
                acc2 = [psD.tile([1, NT], F32, tag=f"rs{j}", name=f"rs{j}", bufs=1)
                        for j in range(nsl)]
                for f in range(FC):
                    sq = tmpd.tile([P, R], F32R, tag="sq2", bufs=2)
                    o1f = out1T[f].bitcast(F32)
                    nc.vector.tensor_mul(out=sq, in0=o1f, in1=o1f)
                    for j in range(nsl):
                        nc.tensor.matmul(acc2[j], ones, sq[:, j * NT:(j + 1) * NT],
                                         start=(f == 0), stop=(f == FC - 1))
                r2_sb = tmpd.tile([1, R], F32, tag="r2sb", bufs=1)
                for j in range(nsl):
                    nc.scalar.activation(out=r2_sb[:, j * NT:(j + 1) * NT],
                                         in_=acc2[j],
                                         func=mybir.ActivationFunctionType.Sqrt,
                                         bias=eps_t, scale=1.0 / D)
                r2r = tmpd.tile([1, R], F32, tag="r2r", bufs=1)
                nc.vector.reciprocal_approx_fast(out=r2r, in_=r2_sb)
                # square: the MLP runs on unnormalized out1; r2^2 commutes to the end
                nc.vector.tensor_mul(out=r2r, in0=r2r, in1=r2r)
                nc.gpsimd.dma_start(out=r2_d, in_=r2r)
                rb2 = mlp.tile([P, R], F32, tag="rb2")
                nc.gpsimd.dma_start(out=rb2, in_=bcast(r2_d))

                partial = [mlp.tile([P, R], F32, tag=f"part{f}", name=f"part{f}")
                           for f in range(FC)]
                gts = [mlp.tile([P, R], F32R, tag=f"g{k}", name=f"g{k}")
                       for k in range(GK)]
                for grp in range(NGRP):
                    for k in range(GK):
                        dmc = grp * GK + k
                        wmblk = ws.tile([P, FC, P], F32R, tag="wm")
                        nc.sync.dma_start(
                            out=wmblk,
                            in_=wm[:, dmc * P:(dmc + 1) * P]
                            .rearrange("(ko p) m -> p ko m", p=P))
                        wnblk = ws.tile([P, FC, P], F32R, tag="wn")
                        nc.sync.dma_start(
                            out=wnblk,
                            in_=wn[:, dmc * P:(dmc + 1) * P]
                            .rearrange("(ko p) m -> p ko m", p=P))
                        for hj in range(R // NT):
                            sl = slice(hj * NT, (hj + 1) * NT)
                            mps = psD.tile([P, NT], F32, tag="mps")
                            nps = psD.tile([P, NT], F32, tag="nps")
                            for f in range(FC):
                                nc.tensor.matmul(mps, wmblk[:, f], out1T[f][:, sl],
                                                 start=(f == 0), stop=(f == FC - 1))
                            for f in range(FC):
                                nc.tensor.matmul(nps, wnblk[:, f], out1T[f][:, sl],
                                                 start=(f == 0), stop=(f == FC - 1))
                            mcp = tmpd.tile([P, NT], F32, tag="mcp")
                            nc.scalar.activation(
                                out=mcp, in_=mps,
                                func=mybir.ActivationFunctionType.Copy,
                                bias=0.0, scale=1.0)
                            nc.vector.tensor_mul(out=gts[k][:, sl], in0=mcp, in1=nps)
                    # wp pass for this group
                    for f in range(FC):
                        wpf = ws.tile([P, GK, P], F32R, tag="wpf")
                        nc.sync.dma_start(
                            out=wpf,
                            in_=wp[grp * GK * P:(grp + 1) * GK * P,
                                   f * P:(f + 1) * P]
                            .rearrange("(ko p) m -> p ko m", p=P))
                        for hj in range(R // NT):
                            sl = slice(hj * NT, (hj + 1) * NT)
                            wps = psD.tile([P, NT], F32, tag="wps")
                            for k in range(GK):
                                nc.tensor.matmul(wps, wpf[:, k], gts[k][:, sl],
                                                 start=(k == 0), stop=(k == GK - 1))
                            if grp == 0:
                                nc.vector.tensor_copy(out=partial[f][:, sl], in_=wps)
                            elif grp < NGRP - 1:
                                nc.vector.tensor_add(out=partial[f][:, sl], in0=wps,
                                                     in1=partial[f][:, sl])
                            else:
                                ot = tmpd.tile([P, NT], F32, tag="ot")
                                nc.vector.tensor_add(out=ot, in0=wps,
                                                     in1=partial[f][:, sl])
                                nc.vector.tensor_mul(out=ot, in0=ot,
                                                     in1=rb2[:, sl])
                                fin = tmpd.tile([P, NT], F32, tag="fin")
                                nc.vector.tensor_add(out=fin, in0=ot,
                                                     in1=out1T[f].bitcast(F32)[:, sl])
                                nc.gpsimd.dma_start(out=outT[f * P:(f + 1) * P, sl],
                                                  in_=fin)

    nc.compile()
    return nc


def _get_program():
    if "nc" not in _cached:
        _cached["nc"] = _build()
    return _cached["nc"]


def kernel(x, cos, sin, causal_mask, wq1, wq2, wk1, wk2, wv, wo, wm, wn, wp):
    global LAST_EXEC_NS
    x = np.asarray(x, dtype=np.float32)
    cos = np.asarray(cos, dtype=np.float32)
    sin = np.asarray(sin, dtype=np.float32)
    causal_mask = np.asarray(causal_mask)
    B = x.shape[0]
    scale = 1.0 / np.sqrt(DH)

    coscat = np.concatenate([cos, cos], axis=1).T.copy()          # [128, S]
    sincat = np.concatenate([-sin, sin], axis=1).T.copy()         # [128, S]
    mask_val = np.where(causal_mask, 0.0, 1.0).astype(np.float32)  # [S, S]

    nc = _get_program()
    in_maps = []
    for c in range(8):
        b, h = c // 2, c % 2
        q0 = h * R
        xb = x[b]
        in_maps.append({
            "xT": np.ascontiguousarray(xb.T),
            "xqT": np.ascontiguousarray(xb[q0:q0 + R].T),
            "cos_kv": coscat,
            "sin_kv": sincat,
            "cos_q": np.ascontiguousarray(coscat[:, q0:q0 + R] * scale),
            "sin_q": np.ascontiguousarray(sincat[:, q0:q0 + R] * scale),
            "maskT": np.ascontiguousarray(mask_val[q0:q0 + R, :].T).astype(ml_dtypes.bfloat16),
            "wq1": np.asarray(wq1, np.float32), "wq2": np.asarray(wq2, np.float32),
            "wk1": np.asarray(wk1, np.float32), "wk2": np.asarray(wk2, np.float32),
            "wv": np.asarray(wv, np.float32), "wo": np.asarray(wo, np.float32),
            "wm": np.asarray(wm, np.float32), "wn": np.asarray(wn, np.float32),
            "wp": np.asarray(wp, np.float32),
        })

    trace = bool(os.environ.get("BASSK_TRACE"))
    if trace:
        _install_trace_hook()
    res = run_bass_kernel_spmd(nc, in_maps, core_ids=list(range(8)), trace=trace)
    LAST_EXEC_NS = res.exec_time_ns

    out = np.empty((B, S, D), dtype=np.float32)
    for c in range(8):
        b, h = c // 2, c % 2
        q0 = h * R
        out[b, q0:q0 + R, :] = res.results[c]["outT"].T
    return out


def _install_trace_hook():
    import types
    import antenv
    if getattr(antenv, "axon_hooks", None) is not None:
        return
    holder = {}
    m = types.ModuleType("antenv.axon_hooks")
    m.set_axon_ntff_profile_hook = lambda h: holder.__setitem__('h', h)
    m.get_axon_ntff_profile_hook = lambda: holder.get('h')
    sys.modules["antenv.axon_hooks"] = m
    antenv.axon_hooks = m
    from trn_agent_boot.trn_boot import _ntff_profile_via_ctypes
    m.set_axon_ntff_profile_hook(_ntff_profile_via_ctypes('/opt/axon/libaxon_pjrt.so'))



# revision 10
# speedup vs baseline: 1.4962x; 1.4962x over previous
"""Trainium2 Bass kernel for nn_BilinearBlock (bilinear attention + bilinear MLP block).

Sharding: 8 cores = (batch b in 0..3) x (sequence half h in 0..1).
Each core computes output rows [h*1024, (h+1)*1024) of batch b.

Everything on-device is kept feature-major ("T layout": features/head-dims on
SBUF partitions, sequence positions on the free axis) so that every matmul
contracts over the partition dim with zero on-device transposes of
activations (only V needs a PE transpose).  RMSNorm is algebraically commuted
past the linear projections: projections run on raw x, and the per-row norm
factor r = rsqrt(mean(x^2)+eps) is applied to the small projected tensors.
The causal mask (an arbitrary 0/1 mask, taken from the causal_mask input) is
applied as a multiply. All matmuls run in float32r (full PE rate, ~2^-13
rounding).
"""
import os
import sys

for _p in ('/opt/trn_rl_repo',):
    if _p not in sys.path:
        sys.path.insert(0, _p)

import numpy as np
import ml_dtypes

import concourse.bass as bass
import concourse.mybir as mybir
import concourse.tile as tile
from concourse import bacc
from concourse.bass_utils import run_bass_kernel_spmd
from concourse.masks import make_identity

P = 128
S = 2048          # full sequence
R = 1024          # query rows per core
D = 1024          # d_model
DH = 128          # d_head
DM = 4096         # d_mlp
NT = 512          # matmul moving free dim
FC = D // P       # 8 feature chunks
TC = S // P       # 16 t chunks
NGRP = 4          # d_mlp groups for the wp pass
GK = DM // P // NGRP  # 8 dm chunks per group
EPS = 1e-6
F32 = mybir.dt.float32
F32R = mybir.dt.float32r

LAST_EXEC_NS = None

_cached = {}


def _build():
    nc = bacc.Bacc("TRN2", target_bir_lowering=False, debug=False, num_devices=8)

    xT = nc.dram_tensor("xT", [D, S], F32R, kind="ExternalInput").ap()
    xqT = nc.dram_tensor("xqT", [D, R], F32R, kind="ExternalInput").ap()
    cos_kv = nc.dram_tensor("cos_kv", [DH, S], F32, kind="ExternalInput").ap()
    sin_kv = nc.dram_tensor("sin_kv", [DH, S], F32, kind="ExternalInput").ap()
    cos_q = nc.dram_tensor("cos_q", [DH, R], F32, kind="ExternalInput").ap()
    sin_q = nc.dram_tensor("sin_q", [DH, R], F32, kind="ExternalInput").ap()
    maskT = nc.dram_tensor("maskT", [S, R], mybir.dt.bfloat16, kind="ExternalInput").ap()
    wq1 = nc.dram_tensor("wq1", [D, DH], F32R, kind="ExternalInput").ap()
    wq2 = nc.dram_tensor("wq2", [D, DH], F32R, kind="ExternalInput").ap()
    wk1 = nc.dram_tensor("wk1", [D, DH], F32R, kind="ExternalInput").ap()
    wk2 = nc.dram_tensor("wk2", [D, DH], F32R, kind="ExternalInput").ap()
    wv = nc.dram_tensor("wv", [D, DH], F32R, kind="ExternalInput").ap()
    wo = nc.dram_tensor("wo", [DH, D], F32R, kind="ExternalInput").ap()
    FP8 = mybir.dt.float8e4
    wm = nc.dram_tensor("wm", [D, DM], FP8, kind="ExternalInput").ap()
    wn = nc.dram_tensor("wn", [D, DM], FP8, kind="ExternalInput").ap()
    wp = nc.dram_tensor("wp", [DM, D], FP8, kind="ExternalInput").ap()
    outT = nc.dram_tensor("outT", [D, R], F32, kind="ExternalOutput").ap()

    # DRAM scratch for broadcasting per-row norm factors across partitions
    rkv_d = nc.dram_tensor("rkv_scratch", [1, S], F32).ap()
    rq_d = nc.dram_tensor("rq_scratch", [1, R], F32).ap()
    r2_d = nc.dram_tensor("r2_scratch", [1, R], F32).ap()

    def bcast(src_dram):
        return bass.AP(tensor=src_dram.tensor, offset=src_dram.offset,
                       ap=[[0, P]] + list(src_dram.ap[1:]))

    with tile.TileContext(nc) as tc:
        with tc.tile_pool(name="glob", bufs=1) as glob, \
             tc.tile_pool(name="tmp", bufs=2) as tmp:

            ident = glob.tile([P, P], F32, tag="ident")
            make_identity(nc, ident)
            ones_f = glob.tile([P, 1], F32, tag="ones_f")
            nc.vector.memset(ones_f, 1.0)
            ones = glob.tile([P, 1], F32R, tag="ones")
            nc.vector.tensor_copy(out=ones, in_=ones_f)
            eps_t = glob.tile([1, 1], F32, tag="eps")
            nc.vector.memset(eps_t, EPS)
            out1T = [glob.tile([P, R], F32R, tag=f"out1T{f}", name=f"out1T{f}")
                     for f in range(FC)]

            with tc.tile_pool(name="attn", bufs=1) as attn:
                k1Tb = [attn.tile([DH, NT], F32R, tag=f"k1T{j}", name=f"k1T{j}")
                        for j in range(S // NT)]
                k2Tb = [attn.tile([DH, NT], F32R, tag=f"k2T{j}", name=f"k2T{j}")
                        for j in range(S // NT)]
                q1Tb = [attn.tile([DH, NT], F32R, tag=f"q1T{j}", name=f"q1T{j}")
                        for j in range(R // NT)]
                q2Tb = [attn.tile([DH, NT], F32R, tag=f"q2T{j}", name=f"q2T{j}")
                        for j in range(R // NT)]
                v_rm = [attn.tile([P, DH], F32R, tag=f"vrm{i}", name=f"vrm{i}")
                        for i in range(TC)]
                attnT = attn.tile([DH, R], F32R, tag="attnT")

                # ================= phase A: projections, block-streamed =========
                with tc.tile_pool(name="xs", bufs=2) as xs, \
                     tc.tile_pool(name="wks", bufs=1) as wks, \
                     tc.tile_pool(name="sc", bufs=2) as sc, \
                     tc.tile_pool(name="psA", bufs=2, space="PSUM") as psA:

                    wblks = {}
                    for nm, w in [("wq1", wq1), ("wq2", wq2), ("wk1", wk1),
                                  ("wk2", wk2), ("wv", wv)]:
                        t = wks.tile([P, FC, DH], F32R, tag=nm, name=nm)
                        nc.gpsimd.dma_start(
                            out=t, in_=w.rearrange("(ko p) m -> p ko m", p=P))
                        wblks[nm] = t

                    def do_block(x_dram, r_dram, cos_d, sin_d, sl, projs, tbase,
                                 xtag="xb", xbufs=2):
                        """Process one 512-column block: norm factor + projections.

                        projs: list of (wname, out_tile or vrm handling, kind)
                        """
                        xr = x_dram.rearrange("(ko p) n -> p ko n", p=P)
                        xb = []
                        for f in range(FC):
                            t = xs.tile([P, NT], F32R, tag=f"{xtag}{f}",
                                        name=f"{xtag}{f}", bufs=xbufs)
                            nc.sync.dma_start(out=t, in_=xr[:, f, sl])
                            xb.append(t)
                        # norm factor for this block
                        rp = psA.tile([1, NT], F32, tag="rp", bufs=1)
                        for f in range(FC):
                            sq = tmp.tile([P, NT], F32R, tag="sqr")
                            sf = xb[f].bitcast(F32)
                            if f % 2 == 0:
                                nc.scalar.activation(
                                    out=sq, in_=sf,
                                    func=mybir.ActivationFunctionType.Square,
                                    bias=0.0, scale=1.0)
                            else:
                                nc.vector.tensor_mul(out=sq, in0=sf, in1=sf)
                            nc.tensor.matmul(rp, ones, sq,
                                             start=(f == 0), stop=(f == FC - 1))
                        rsb = tmp.tile([1, NT], F32, tag="rsb")
                        nc.scalar.activation(out=rsb, in_=rp,
                                             func=mybir.ActivationFunctionType.Sqrt,
                                             bias=eps_t, scale=1.0 / D)
                        rsb2 = tmp.tile([1, NT], F32, tag="rsb2")
                        nc.vector.reciprocal_approx_fast(out=rsb2, in_=rsb)
                        nc.gpsimd.dma_start(out=r_dram[:, sl], in_=rsb2)
                        rbb = xs.tile([P, NT], F32, tag="rbb")
                        nc.gpsimd.dma_start(out=rbb, in_=bcast(r_dram[:, sl]))
                        # rope tables for this block
                        cosb = xs.tile([DH, NT], F32, tag="cosb")
                        nc.sync.dma_start(out=cosb, in_=cos_d[:, sl])
                        sinb = xs.tile([DH, NT], F32, tag="sinb")
                        nc.sync.dma_start(out=sinb, in_=sin_d[:, sl])

                        for wname, dst, kind in projs:
                            pp = psA.tile([P, NT], F32, tag="pp", bufs=4)
                            wb = wblks[wname]
                            for f in range(FC):
                                nc.tensor.matmul(pp, wb[:, f], xb[f],
                                                 start=(f == 0), stop=(f == FC - 1))
                            if kind == "rope":
                                t1 = tmp.tile([P, NT], F32, tag="t1")
                                nc.vector.tensor_mul(out=t1, in0=pp, in1=cosb)
                                rot = tmp.tile([P, NT], F32, tag="rot")
                                nc.scalar.activation(
                                    out=rot[0:64], in_=pp[64:128],
                                    func=mybir.ActivationFunctionType.Copy,
                                    bias=0.0, scale=1.0)
                                nc.scalar.activation(
                                    out=rot[64:128], in_=pp[0:64],
                                    func=mybir.ActivationFunctionType.Copy,
                                    bias=0.0, scale=1.0)
                                nc.vector.tensor_mul(out=rot, in0=rot, in1=sinb)
                                nc.vector.tensor_add(out=t1, in0=t1, in1=rot)
                                nc.vector.tensor_mul(out=dst, in0=t1, in1=rbb)
                            else:  # v: scale + transpose to row-major blocks
                                vt = tmp.tile([P, NT], F32, tag="t1")
                                nc.vector.tensor_mul(out=vt, in0=pp, in1=rbb)
                                for t in range(NT // P):
                                    tp = psA.tile([P, P], F32, tag="tp", bufs=1)
                                    nc.tensor.transpose(tp, vt[:, t * P:(t + 1) * P],
                                                        ident)
                                    nc.scalar.activation(
                                        out=v_rm[tbase + t], in_=tp,
                                        func=mybir.ActivationFunctionType.Copy,
                                        bias=0.0, scale=1.0)

                    for jb in range(R // NT):
                        sl = slice(jb * NT, (jb + 1) * NT)
                        do_block(xqT, rq_d, cos_q, sin_q, sl,
                                 [("wq1", q1Tb[jb], "rope"),
                                  ("wq2", q2Tb[jb], "rope")],
                                 tbase=0, xtag="xq", xbufs=2)
                    for jb in range(S // NT):
                        sl = slice(jb * NT, (jb + 1) * NT)
                        do_block(xT, rkv_d, cos_kv, sin_kv, sl,
                                 [("wk1", k1Tb[jb], "rope"),
                                  ("wk2", k2Tb[jb], "rope"),
                                  ("wv", None, "v")], tbase=jb * (NT // P))

                    # ---- scores + attn@v, interleaved with projections ----
                    avp = [psA.tile([P, NT], F32, tag=f"av{hj}", name=f"av{hj}",
                                    bufs=1)
                           for hj in range(R // NT)]
                    for i in range(TC):
                        mk = sc.tile([P, R], mybir.dt.bfloat16, tag="mk")
                        nc.sync.dma_start(out=mk, in_=maskT[i * P:(i + 1) * P, :])
                        kb, ko = i // 4, (i % 4) * P
                        for hj in range(R // NT):
                            s1 = psA.tile([P, NT], F32, tag="pp", name="s1", bufs=4)
                            nc.tensor.matmul(s1, k1Tb[kb][:, ko:ko + P],
                                             q1Tb[hj], start=True, stop=True)
                            s2 = psA.tile([P, NT], F32, tag="pp", name="s2", bufs=4)
                            nc.tensor.matmul(s2, k2Tb[kb][:, ko:ko + P],
                                             q2Tb[hj], start=True, stop=True)
                            sm = tmp.tile([P, NT], F32, tag="sm", bufs=3)
                            nc.vector.tensor_mul(out=sm, in0=s1,
                                                 in1=mk[:, hj * NT:(hj + 1) * NT])
                            aT = sc.tile([P, NT], F32R, tag="aT", bufs=4)
                            nc.vector.tensor_mul(out=aT, in0=sm, in1=s2)
                            nc.tensor.matmul(avp[hj], v_rm[i], aT,
                                             start=(i == 0), stop=(i == TC - 1))
                    for hj in range(R // NT):
                        nc.vector.tensor_copy(
                            out=attnT[:, hj * NT:(hj + 1) * NT], in_=avp[hj])


                # ============ phase C: out1 = x + attn @ wo ====================
                with tc.tile_pool(name="oc", bufs=2) as oc, \
                     tc.tile_pool(name="psC", bufs=2, space="PSUM") as psC:
                    woblk = oc.tile([P, FC, P], F32R, tag="wo", bufs=1)
                    nc.gpsimd.dma_start(
                        out=woblk, in_=wo.rearrange("d (ko m) -> d ko m", m=P))
                    for f in range(FC):
                        xqr = oc.tile([P, R], F32, tag="xqr")
                        nc.gpsimd.dma_start(
                            out=xqr, in_=xqT.bitcast(F32)[f * P:(f + 1) * P, :])
                        for hj in range(R // NT):
                            sl = slice(hj * NT, (hj + 1) * NT)
                            pw = psC.tile([P, NT], F32, tag="pw")
                            nc.tensor.matmul(pw, woblk[:, f], attnT[:, sl],
                                             start=True, stop=True)
                            nc.vector.tensor_add(out=out1T[f][:, sl], in0=pw,
                                                 in1=xqr[:, sl])

            # ============ phase D: rmsnorm2 + bilinear MLP (fp8 DoubleRow) =====
            # wm/wn/wp arrive pre-scaled by 8 in fp8e4.  xn2 = out1 * r2 is
            # quantized to fp8 on the fly; h1 is copied to bf16 at true scale,
            # gts = h1 * (8*h2) carries x8; the wp result carries x64, undone
            # in the final PSUM->SBUF copy.
            DR = mybir.MatmulPerfMode.DoubleRow
            nsl = R // NT
            with tc.tile_pool(name="mlp", bufs=1) as mlp, \
                 tc.tile_pool(name="ws", bufs=2) as ws, \
                 tc.tile_pool(name="tmpd", bufs=2) as tmpd:

                xn8h = [mlp.tile([P, FC, NT], FP8, tag=f"xn8_{j}", name=f"xn8_{j}")
                        for j in range(nsl)]
                with tc.tile_pool(name="psR", bufs=1, space="PSUM") as psR:
                    for j in range(nsl):
                        slj = slice(j * NT, (j + 1) * NT)
                        acc = psR.tile([1, NT], F32, tag="rs", bufs=2)
                        for f in range(FC):
                            sq = tmpd.tile([P, NT], F32R, tag="sq2", bufs=2)
                            nc.scalar.activation(
                                out=sq, in_=out1T[f].bitcast(F32)[:, slj],
                                func=mybir.ActivationFunctionType.Square,
                                bias=0.0, scale=1.0)
                            nc.tensor.matmul(acc, ones, sq,
                                             start=(f == 0), stop=(f == FC - 1))
                        r2s = tmpd.tile([1, NT], F32, tag="r2sb", bufs=2)
                        nc.scalar.activation(out=r2s, in_=acc,
                                             func=mybir.ActivationFunctionType.Sqrt,
                                             bias=eps_t, scale=1.0 / D)
                        r2r = tmpd.tile([1, NT], F32, tag="r2r", bufs=2)
                        nc.vector.reciprocal_approx_fast(out=r2r, in_=r2s)
                        rb2 = mlp.tile([P, NT], F32, tag=f"rb2_{j}",
                                       name=f"rb2_{j}")
                        nc.gpsimd.partition_broadcast(rb2, r2r)
                        for f in range(FC):
                            nc.vector.tensor_mul(out=xn8h[j][:, f],
                                                 in0=out1T[f].bitcast(F32)[:, slj],
                                                 in1=rb2)

                gts = mlp.tile([P, DM // P, R], FP8, tag="gts")
                with tc.tile_pool(name="psD", bufs=2, space="PSUM") as psD:
                    for dmc in range(DM // P):
                        wmblk = ws.tile([P, FC, P], FP8, tag="wm")
                        nc.sync.dma_start(
                            out=wmblk,
                            in_=wm[:, dmc * P:(dmc + 1) * P]
                            .rearrange("(ko p) m -> p ko m", p=P))
                        wnblk = ws.tile([P, FC, P], FP8, tag="wn")
                        nc.sync.dma_start(
                            out=wnblk,
                            in_=wn[:, dmc * P:(dmc + 1) * P]
                            .rearrange("(ko p) m -> p ko m", p=P))
                        # hj pairs share each stationary weight load
                        mps = [psD.tile([P, NT], F32, tag="mps", bufs=3,
                                        name=f"mps{hj}")
                               for hj in range(nsl)]
                        nps = [psD.tile([P, NT], F32, tag="nps", bufs=3,
                                        name=f"nps{hj}")
                               for hj in range(nsl)]
                        for c in range(FC // 2):
                            for hj in range(nsl):
                                nc.tensor.matmul(mps[hj],
                                                 wmblk[:, 2 * c:2 * c + 2],
                                                 xn8h[hj][:, 2 * c:2 * c + 2],
                                                 start=(c == 0),
                                                 stop=(c == FC // 2 - 1),
                                                 perf_mode=DR)
                        for c in range(FC // 2):
                            for hj in range(nsl):
                                nc.tensor.matmul(nps[hj],
                                                 wnblk[:, 2 * c:2 * c + 2],
                                                 xn8h[hj][:, 2 * c:2 * c + 2],
                                                 start=(c == 0),
                                                 stop=(c == FC // 2 - 1),
                                                 perf_mode=DR)
                        for hj in range(nsl):
                            sl = slice(hj * NT, (hj + 1) * NT)
                            mcp = tmpd.tile([P, NT], mybir.dt.bfloat16, tag="mcp")
                            nc.scalar.activation(
                                out=mcp, in_=mps[hj],
                                func=mybir.ActivationFunctionType.Copy,
                                bias=0.0, scale=0.125)
                            nc.vector.tensor_mul(out=gts[:, dmc, sl],
                                                 in0=mcp, in1=nps[hj])

                    # wp pass: accumulate all 32 dm chunks (16 DR matmuls) in
                    # PSUM per (f, hj) output tile
                    with tc.tile_pool(name="psW", bufs=1, space="PSUM") as psW:
                        for f in range(FC):
                            wpf = ws.tile([P, DM // P, P], FP8, tag="wpf")
                            nc.sync.dma_start(
                                out=wpf,
                                in_=wp[:, f * P:(f + 1) * P]
                                .rearrange("(ko p) m -> p ko m", p=P))
                            wps = [psW.tile([P, NT], F32, tag="wps", bufs=2,
                                            name=f"wps{hj}")
                                   for hj in range(nsl)]
                            for c in range(DM // P // 2):
                                for hj in range(nsl):
                                    sl = slice(hj * NT, (hj + 1) * NT)
                                    nc.tensor.matmul(
                                        wps[hj], wpf[:, 2 * c:2 * c + 2],
                                        gts[:, 2 * c:2 * c + 2, sl],
                                        start=(c == 0),
                                        stop=(c == DM // P // 2 - 1),
                                        perf_mode=DR)
                            for hj in range(nsl):
                                sl = slice(hj * NT, (hj + 1) * NT)
                                mlpt = tmpd.tile([P, NT], F32, tag="mlpt")
                                nc.scalar.activation(
                                    out=mlpt, in_=wps[hj],
                                    func=mybir.ActivationFunctionType.Copy,
                                    bias=0.0, scale=1.0 / 64.0)
                                fin = tmpd.tile([P, NT], F32, tag="fin")
                                nc.vector.tensor_add(
                                    out=fin, in0=mlpt,
                                    in1=out1T[f].bitcast(F32)[:, sl])
                                nc.gpsimd.dma_start(
                                    out=outT[f * P:(f + 1) * P, sl], in_=fin)

    nc.compile()
    return nc


def _get_program():
    if "nc" not in _cached:
        _cached["nc"] = _build()
    return _cached["nc"]


def kernel(x, cos, sin, causal_mask, wq1, wq2, wk1, wk2, wv, wo, wm, wn, wp):
    global LAST_EXEC_NS
    x = np.asarray(x, dtype=np.float32)
    cos = np.asarray(cos, dtype=np.float32)
    sin = np.asarray(sin, dtype=np.float32)
    causal_mask = np.asarray(causal_mask)
    B = x.shape[0]
    scale = 1.0 / np.sqrt(DH)

    coscat = np.concatenate([cos, cos], axis=1).T.copy()          # [128, S]
    sincat = np.concatenate([-sin, sin], axis=1).T.copy()         # [128, S]
    mask_val = np.where(causal_mask, 0.0, 1.0).astype(np.float32)  # [S, S]

    def to8(a):
        return np.clip(np.asarray(a, np.float32) * 8.0, -240.0, 240.0).astype(
            ml_dtypes.float8_e4m3)
    wm8, wn8, wp8 = to8(wm), to8(wn), to8(wp)

    nc = _get_program()
    in_maps = []
    for c in range(8):
        b, h = c // 2, c % 2
        q0 = h * R
        xb = x[b]
        in_maps.append({
            "xT": np.ascontiguousarray(xb.T),
            "xqT": np.ascontiguousarray(xb[q0:q0 + R].T),
            "cos_kv": coscat,
            "sin_kv": sincat,
            "cos_q": np.ascontiguousarray(coscat[:, q0:q0 + R] * scale),
            "sin_q": np.ascontiguousarray(sincat[:, q0:q0 + R] * scale),
            "maskT": np.ascontiguousarray(mask_val[q0:q0 + R, :].T).astype(ml_dtypes.bfloat16),
            "wq1": np.asarray(wq1, np.float32), "wq2": np.asarray(wq2, np.float32),
            "wk1": np.asarray(wk1, np.float32), "wk2": np.asarray(wk2, np.float32),
            "wv": np.asarray(wv, np.float32), "wo": np.asarray(wo, np.float32),
            "wm": wm8, "wn": wn8, "wp": wp8,
        })

    trace = bool(os.environ.get("BASSK_TRACE"))
    if trace:
        _install_trace_hook()
    res = run_bass_kernel_spmd(nc, in_maps, core_ids=list(range(8)), trace=trace)
    LAST_EXEC_NS = res.exec_time_ns

    out = np.empty((B, S, D), dtype=np.float32)
    for c in range(8):
        b, h = c // 2, c % 2
        q0 = h * R
        out[b, q0:q0 + R, :] = res.results[c]["outT"].T
    return out


def _install_trace_hook():
    import types
    import antenv
    if getattr(antenv, "axon_hooks", None) is not None:
        return
    holder = {}
    m = types.ModuleType("antenv.axon_hooks")
    m.set_axon_ntff_profile_hook = lambda h: holder.__setitem__('h', h)
    m.get_axon_ntff_profile_hook = lambda: holder.get('h')
    sys.modules["antenv.axon_hooks"] = m
    antenv.axon_hooks = m
    from trn_agent_boot.trn_boot import _ntff_profile_via_ctypes
    m.set_axon_ntff_profile_hook(_ntff_profile_via_ctypes('/opt/axon/libaxon_pjrt.so'))

